# revision 1
# baseline (speedup 1.0000x reference)
"""Trainium2 Bass kernel for nn_CrossAttnBlock (B=4, Lq=Lk=2048, D=1024, H=16, Dh=64).

Sharding: 8 cores = (batch b in 0..3) x (query half in 0..1). Each core computes
cross-attention for 1024 query tokens of one batch against that batch's full
2048-token context. K/V work is duplicated across the two cores of a batch
(zero collectives needed).

Per-core dataflow (all matmuls in fp32r, PE 1 cyc/row):
  P0: LayerNorm x, ctx token-major; PE-transpose to feature-major. qinT stays
      in SBUF; kvT (8 KB/partition) is staged out to a DRAM scratch and
      streamed back per quarter (SBUF is the binding constraint).
  P1: q = qinT.T @ Wq (token-major), l2-normalize rows per head, transpose to
      feature-major qhatT.
  P2: per head-quarter: k/v projections (token-major), k l2-normalized and
      transposed per head to khatT; V'' = [v*mask | mask] built at evict.
      Per head: S^T[k,q] = khat.T @ qhat; exp(S^T/tau') on ACT; O' = V''.T @
      expS^T accumulated over k tiles -> [65, q] with row 64 = softmax
      denominator (the mask column makes denominators exclude masked keys);
      normalize columns by 1/denom into feature-major oaT.
  P3: out = oaT.T @ Wo (token-major), DMA to DRAM.

rsqrt is computed as exp(-0.5*ln(s)) + one Newton step so the whole kernel
needs a single ACT table set (natural_log_exp); ACT Rsqrt/Reciprocal are
banned for accuracy in this bass version. Softmax needs no max-subtraction:
q,k are l2-normalized so |scores/tau'| <= 8.0001.
"""

import contextlib

import numpy as np

import bass_rust
import concourse.bass as bass
import concourse.tile as tile
from concourse import mybir
from concourse.bass_utils import run_bass_kernel_spmd
from concourse.masks import make_identity

F32 = mybir.dt.float32
F32R = mybir.dt.float32r
BF16 = mybir.dt.bfloat16
AF = mybir.ActivationFunctionType
ALU = mybir.AluOpType

B, LQ, LK = 4, 2048, 2048
DQ, DC = 1024, 1024
H, DH = 16, 64
INNER = H * DH
LN_EPS = 1e-5

LQS = LQ // 2          # query tokens per core (1024)
NQT = LQS // 128       # 8 query token tiles
NKT = LK // 128        # 16 k token tiles
NF = DQ // 128         # 8 feature tiles
NQUARTER = 4           # head quarters
HPQ = H // NQUARTER    # 4 heads per quarter


def _rsqrt(nc, pool, s_ap, out_ap, newton=True):
    """out = 1/sqrt(s) via exp(-0.5*ln(s)), optionally one Newton step.

    s_ap must be > 0. Shapes of s_ap/out_ap must match ([P, n])."""
    p, n = s_ap.shape[0], s_ap.free_size()
    t = pool.tile([p, n], F32, tag="rsq_t")
    nc.scalar.activation(out=t[:, :n], in_=s_ap, func=AF.Ln)
    nc.scalar.activation(out=out_ap, in_=t[:, :n], func=AF.Exp, scale=-0.5)
    if newton:
        # r = r * (1.5 - 0.5 * s * r^2)
        a = pool.tile([p, n], F32, tag="rsq_a")
        nc.vector.tensor_mul(out=a[:, :n], in0=out_ap, in1=out_ap)
        nc.vector.tensor_mul(out=a[:, :n], in0=a[:, :n], in1=s_ap)
        nc.vector.tensor_scalar(
            out=a[:, :n], in0=a[:, :n], scalar1=-0.5, scalar2=1.5,
            op0=ALU.mult, op1=ALU.add,
        )
        nc.vector.tensor_mul(out=out_ap, in0=out_ap, in1=a[:, :n])


def _layernorm_tile(nc, pool, x_ap, z_ap):
    """z = (x - mean) * rsqrt(var + eps) for a [128, 1024] token-major tile."""
    p = x_ap.shape[0]
    stats = pool.tile([p, 2, 6], F32, tag="ln_stats")
    x3 = x_ap.rearrange("p (c f) -> p c f", c=2)
    for c in range(2):
        nc.vector.bn_stats(out=stats[:, c, :], in_=x3[:, c, :])
    mv = pool.tile([p, 2], F32, tag="ln_mv")
    nc.vector.bn_aggr(out=mv[:], in_=stats[:])
    s = pool.tile([p, 1], F32, tag="ln_s")
    nc.vector.tensor_scalar(
        out=s[:], in0=mv[:, 1:2], scalar1=LN_EPS, scalar2=None, op0=ALU.add,
    )
    inv = pool.tile([p, 1], F32, tag="ln_inv")
    _rsqrt(nc, pool, s[:], inv[:])
    nc.vector.tensor_scalar(
        out=z_ap, in0=x_ap, scalar1=mv[:, 0:1], scalar2=inv[:],
        op0=ALU.subtract, op1=ALU.mult,
    )


def build_program(inv_tau: float, has_bias: bool):
    """Build the single-core SPMD bass program."""
    nc = bass.Bass()

    xs = nc.declare_dram_parameter("xs", [LQS, DQ], F32, isOutput=False)
    ctx = nc.declare_dram_parameter("ctx", [LK, DC], F32, isOutput=False)
    mask01 = nc.declare_dram_parameter("mask01", [LK], F32, isOutput=False)
    wq = nc.declare_dram_parameter("wq", [DQ, INNER], F32R, isOutput=False)
    wk = nc.declare_dram_parameter("wk", [DC, INNER], F32R, isOutput=False)
    wv = nc.declare_dram_parameter("wv", [DC, INNER], F32R, isOutput=False)
    wo = nc.declare_dram_parameter("wo", [INNER, DQ], F32R, isOutput=False)
    if has_bias:
        cq = nc.declare_dram_parameter("cq", [INNER], F32, isOutput=False)
        ck = nc.declare_dram_parameter("ck", [INNER], F32, isOutput=False)
        cv = nc.declare_dram_parameter("cv", [INNER], F32, isOutput=False)
    out = nc.declare_dram_parameter("out", [LQS, DQ], F32, isOutput=True)

    # feature-major LayerNorm'd context staged in DRAM: [NF, 128, LK]
    kvT_d = nc.dram_tensor("kvT_scratch", [NF, 128, LK], F32R,
                           kind="ExternalOutput")

    with tile.TileContext(nc) as tc:
        with contextlib.ExitStack() as stack:
            consts = stack.enter_context(tc.tile_pool(name="consts", bufs=1))
            ident_f = consts.tile([128, 128], F32)
            make_identity(nc, ident_f[:])
            ident = consts.tile([128, 128], F32R)
            nc.vector.tensor_copy(out=ident[:], in_=ident_f[:])
            identr = ident[:]
            ones_f = consts.tile([1, DH], F32)
            nc.vector.memset(ones_f[:], 1.0)
            ones_dh = consts.tile([1, DH], F32R)
            nc.vector.tensor_copy(out=ones_dh[:], in_=ones_f[:])

            # mask as [128, NKT] float (token t*128+p at [p, t])
            mask_sb = consts.tile([128, NKT], F32)
            nc.sync.dma_start(
                out=mask_sb[:], in_=mask01.rearrange("(t p) -> p t", p=128)
            )
            if has_bias:
                # per-out-feature projection bias rows broadcast to 128 parts
                cq_b = consts.tile([128, INNER], F32)
                ck_b = consts.tile([128, INNER], F32)
                cv_b = consts.tile([128, INNER], F32)
                for dst, src in ((cq_b, cq), (ck_b, ck), (cv_b, cv)):
                    bc = bass.AP(
                        tensor=src.tensor, offset=src.offset,
                        ap=[[0, 128]] + list(src.ap),
                    )
                    nc.gpsimd.dma_start(out=dst[:], in_=bc)

            small = stack.enter_context(tc.tile_pool(name="small", bufs=2))

            # persistent feature-major activations (32 KB/part total)
            persist = stack.enter_context(tc.tile_pool(name="persist", bufs=1))
            qhatT = [
                persist.tile([128, LQS], F32R, name=f"qhatT{i}", tag=f"qhatT{i}")
                for i in range(NF)
            ]
            oaT = [
                persist.tile([128, LQS], F32R, name=f"oaT{i}", tag=f"oaT{i}")
                for i in range(NF)
            ]

            # qinT lives P0 -> P1 (its own pool so it frees before P2)
            with contextlib.ExitStack() as qin_era:
                qin_pool = qin_era.enter_context(
                    tc.tile_pool(name="qin", bufs=1)
                )
                qinT = [
                    qin_pool.tile([128, LQS], F32R, name=f"qinT{i}",
                                  tag=f"qinT{i}")
                    for i in range(NF)
                ]

                # ---- P0 ----
                with contextlib.ExitStack() as p0:
                    ln_pool = p0.enter_context(tc.tile_pool(name="ln", bufs=3))
                    tp_psum = p0.enter_context(
                        tc.tile_pool(name="tp_psum0", bufs=4, space="PSUM")
                    )

                    for t in range(NQT):
                        xt = ln_pool.tile([128, DQ], F32, tag="ln_x")
                        nc.sync.dma_start(
                            out=xt[:], in_=xs[t * 128:(t + 1) * 128, :]
                        )
                        zt = ln_pool.tile([128, DQ], F32R, tag="ln_z")
                        _layernorm_tile(nc, small, xt[:], zt[:])
                        for fi in range(NF):
                            tp = tp_psum.tile([128, 128], F32, tag="tp")
                            nc.tensor.transpose(
                                tp[:].bitcast(F32R),
                                zt[:, fi * 128:(fi + 1) * 128].bitcast(F32R),
                                identr,
                            )
                            nc.vector.tensor_copy(
                                out=qinT[fi][:, t * 128:(t + 1) * 128],
                                in_=tp[:],
                            )

                    # ctx: token-tile pairs; write kvT to DRAM in [128, 256]
                    # chunks so DMA lines are 1 KB
                    stg_pool = p0.enter_context(
                        tc.tile_pool(name="kvstg", bufs=3)
                    )
                    for tg in range(NKT // 2):
                        zts = []
                        for ti in range(2):
                            t = 2 * tg + ti
                            xt = ln_pool.tile([128, DQ], F32, tag="ln_x")
                            nc.sync.dma_start(
                                out=xt[:], in_=ctx[t * 128:(t + 1) * 128, :]
                            )
                            zt = ln_pool.tile([128, DQ], F32R, tag=f"ln_zc{ti}")
                            _layernorm_tile(nc, small, xt[:], zt[:])
                            zts.append(zt)
                        for fi in range(NF):
                            stg = stg_pool.tile([128, 256], F32R, tag="kvstg")
                            for ti in range(2):
                                tp = tp_psum.tile([128, 128], F32, tag="tp")
                                nc.tensor.transpose(
                                    tp[:].bitcast(F32R),
                                    zts[ti][:, fi * 128:(fi + 1) * 128
                                            ].bitcast(F32R),
                                    identr,
                                )
                                nc.vector.tensor_copy(
                                    out=stg[:, ti * 128:(ti + 1) * 128],
                                    in_=tp[:],
                                )
                            nc.sync.dma_start(
                                out=kvT_d[fi, :, tg * 256:(tg + 1) * 256],
                                in_=stg[:],
                            )

                # ---- P1: q projection, l2 norm, transpose ----
                with contextlib.ExitStack() as p1:
                    tp_psum = p1.enter_context(
                        tc.tile_pool(name="tp_psum1", bufs=2, space="PSUM")
                    )
                    mm_psum = p1.enter_context(
                        tc.tile_pool(name="mm_psum1", bufs=3, space="PSUM")
                    )
                    wq_pool = p1.enter_context(tc.tile_pool(name="wq", bufs=1))
                    wq_sb = wq_pool.tile([128, 2, NF, 512], F32R)
                    for fi in range(NF):
                        for n in range(2):
                            nc.sync.dma_start(
                                out=wq_sb[:, n, fi, :],
                                in_=wq[fi * 128:(fi + 1) * 128,
                                       n * 512:(n + 1) * 512],
                            )
                    qt_pool = p1.enter_context(tc.tile_pool(name="qtok", bufs=2))
                    for t in range(NQT):
                        qtok = qt_pool.tile([128, INNER], F32R, tag="qtok")
                        for n in range(2):
                            ps = mm_psum.tile([128, 512], F32, tag="mm")
                            for fi in range(NF):
                                nc.tensor.matmul(
                                    ps[:],
                                    qinT[fi][:, t * 128:(t + 1) * 128
                                             ].bitcast(F32R),
                                    wq_sb[:, n, fi, :].bitcast(F32R),
                                    start=(fi == 0),
                                    stop=(fi == NF - 1),
                                )
                            if has_bias:
                                nc.vector.tensor_add(
                                    out=qtok[:, n * 512:(n + 1) * 512],
                                    in0=ps[:],
                                    in1=cq_b[:, n * 512:(n + 1) * 512],
                                )
                            else:
                                nc.vector.tensor_copy(
                                    out=qtok[:, n * 512:(n + 1) * 512],
                                    in_=ps[:],
                                )
                        # ssq per head -> rsqrt -> normalize in place
                        sq = qt_pool.tile([128, INNER], F32, tag="qsq")
                        nc.vector.scalar_tensor_tensor(
                            out=sq[:], in0=qtok[:], scalar=1.0, in1=qtok[:],
                            op0=ALU.mult, op1=ALU.mult,
                        )
                        ssq = small.tile([128, H], F32, tag="qssq")
                        nc.vector.tensor_reduce(
                            out=ssq[:],
                            in_=sq[:].rearrange("p (h d) -> p h d", h=H),
                            axis=mybir.AxisListType.X,
                            op=ALU.add,
                        )
                        rq = small.tile([128, H], F32, tag="qrq")
                        _rsqrt(nc, small, ssq[:], rq[:])
                        q3 = qtok[:].rearrange("p (h d) -> p h d", h=H)
                        nc.vector.tensor_tensor(
                            out=q3,
                            in0=q3,
                            in1=rq[:].unsqueeze(2).broadcast_to([128, H, DH]),
                            op=ALU.mult,
                        )
                        for fi in range(NF):
                            tp = tp_psum.tile([128, 128], F32, tag="tp")
                            nc.tensor.transpose(
                                tp[:].bitcast(F32R),
                                qtok[:, fi * 128:(fi + 1) * 128].bitcast(F32R),
                                identr,
                            )
                            nc.vector.tensor_copy(
                                out=qhatT[fi][:, t * 128:(t + 1) * 128],
                                in_=tp[:],
                            )

            # ---- P2: per-quarter k/v projection + attention ----
            for Q in range(NQUARTER):
                with contextlib.ExitStack() as p2:
                    wkv_pool = p2.enter_context(
                        tc.tile_pool(name=f"wkv{Q}", bufs=1)
                    )
                    wk_sb = wkv_pool.tile([128, NF, 256], F32R, name=f"wk_sb{Q}")
                    wv_sb = wkv_pool.tile([128, NF, 256], F32R, name=f"wv_sb{Q}")
                    c0 = Q * 256
                    for fi in range(NF):
                        nc.sync.dma_start(
                            out=wk_sb[:, fi, :],
                            in_=wk[fi * 128:(fi + 1) * 128, c0:c0 + 256],
                        )
                        nc.sync.dma_start(
                            out=wv_sb[:, fi, :],
                            in_=wv[fi * 128:(fi + 1) * 128, c0:c0 + 256],
                        )

                    khatT = [
                        p2.enter_context(
                            tc.tile_pool(name=f"khatT{Q}_{i}", bufs=1)
                        ).tile([128, LK], F32R, name=f"khatT{Q}_{i}t")
                        for i in range(HPQ // 2)
                    ]
                    vpp_pool = p2.enter_context(
                        tc.tile_pool(name=f"vpp{Q}", bufs=1)
                    )
                    vpp = vpp_pool.tile([128, NKT, HPQ, DH + 1], F32R,
                                        name=f"vpp{Q}t")
                    kt_pool = p2.enter_context(
                        tc.tile_pool(name=f"ktok{Q}", bufs=3)
                    )

                    with contextlib.ExitStack() as kvps:
                        mm_psum = kvps.enter_context(
                            tc.tile_pool(name=f"mm_psum2_{Q}", bufs=2,
                                         space="PSUM")
                        )
                        tp_psum = kvps.enter_context(
                            tc.tile_pool(name=f"tp_psum2_{Q}", bufs=2,
                                         space="PSUM")
                        )
                        kvs_pool = kvps.enter_context(
                            tc.tile_pool(name=f"kvs{Q}", bufs=3)
                        )
                        for tg in range(NKT // 2):
                            kvs = kvs_pool.tile([128, NF, 256], F32R, tag="kvs")
                            for fi in range(NF):
                                nc.sync.dma_start(
                                    out=kvs[:, fi, :],
                                    in_=kvT_d[fi, :, tg * 256:(tg + 1) * 256],
                                )
                            for ti in range(2):
                                t = 2 * tg + ti
                                ps_k = mm_psum.tile([128, 256], F32, tag="mm_k")
                                ps_v = mm_psum.tile([128, 256], F32, tag="mm_v")
                                for fi in range(NF):
                                    lhs = kvs[:, fi, ti * 128:(ti + 1) * 128
                                              ].bitcast(F32R)
                                    nc.tensor.matmul(
                                        ps_k[:], lhs,
                                        wk_sb[:, fi, :].bitcast(F32R),
                                        start=(fi == 0), stop=(fi == NF - 1),
                                    )
                                    nc.tensor.matmul(
                                        ps_v[:], lhs,
                                        wv_sb[:, fi, :].bitcast(F32R),
                                        start=(fi == 0), stop=(fi == NF - 1),
                                    )
                                ktok = kt_pool.tile([128, 256], F32R, tag="ktok")
                                if has_bias:
                                    nc.vector.tensor_add(
                                        out=ktok[:], in0=ps_k[:],
                                        in1=ck_b[:, c0:c0 + 256],
                                    )
                                else:
                                    nc.vector.tensor_copy(
                                        out=ktok[:], in_=ps_k[:]
                                    )
                                # V'' = [v * mask | mask]
                                mt = mask_sb[:, t:t + 1]
                                if has_bias:
                                    vtmp = kt_pool.tile([128, 256], F32,
                                                        tag="vtmp")
                                    nc.vector.tensor_add(
                                        out=vtmp[:], in0=ps_v[:],
                                        in1=cv_b[:, c0:c0 + 256],
                                    )
                                    vsrc = vtmp[:]
                                else:
                                    vsrc = ps_v[:]
                                nc.vector.tensor_scalar_mul(
                                    out=vpp[:, t, :, 0:DH],
                                    in0=vsrc.rearrange(
                                        "p (h d) -> p h d", h=HPQ
                                    ),
                                    scalar1=mt,
                                )
                                for h in range(HPQ):
                                    nc.vector.tensor_copy(
                                        out=vpp[:, t, h, DH:DH + 1], in_=mt
                                    )
                                # k l2 norm
                                sqk = kt_pool.tile([128, 256], F32, tag="ksq")
                                nc.vector.scalar_tensor_tensor(
                                    out=sqk[:], in0=ktok[:], scalar=1.0,
                                    in1=ktok[:], op0=ALU.mult, op1=ALU.mult,
                                )
                                ssqk = small.tile([128, HPQ], F32, tag="kssq")
                                nc.vector.tensor_reduce(
                                    out=ssqk[:],
                                    in_=sqk[:].rearrange(
                                        "p (h d) -> p h d", h=HPQ
                                    ),
                                    axis=mybir.AxisListType.X,
                                    op=ALU.add,
                                )
                                rk = small.tile([128, HPQ], F32, tag="krk")
                                _rsqrt(nc, small, ssqk[:], rk[:])
                                k3 = ktok[:].rearrange("p (h d) -> p h d",
                                                       h=HPQ)
                                nc.vector.tensor_tensor(
                                    out=k3,
                                    in0=k3,
                                    in1=rk[:].unsqueeze(2).broadcast_to(
                                        [128, HPQ, DH]
                                    ),
                                    op=ALU.mult,
                                )
                                # transpose each head's [128, 64] -> [64, 128]
                                for h in range(HPQ):
                                    tp = tp_psum.tile([128, 128], F32,
                                                      tag="tp")
                                    nc.tensor.transpose(
                                        tp[0:DH, :].bitcast(F32R),
                                        ktok[:, h * DH:(h + 1) * DH
                                             ].bitcast(F32R),
                                        identr,
                                    )
                                    r0 = (h % 2) * DH
                                    nc.vector.tensor_copy(
                                        out=khatT[h // 2][
                                            r0:r0 + DH, t * 128:(t + 1) * 128
                                        ],
                                        in_=tp[0:DH, :],
                                    )

                    # attention for the 4 heads of this quarter
                    exp_pool = p2.enter_context(
                        tc.tile_pool(name=f"exp{Q}", bufs=4)
                    )
                    rec_pool = p2.enter_context(
                        tc.tile_pool(name=f"rec{Q}", bufs=2)
                    )

                    s_psum = p2.enter_context(
                        tc.tile_pool(name=f"s_psum{Q}", bufs=2, space="PSUM")
                    )
                    ps_o_pool = p2.enter_context(
                        tc.tile_pool(name=f"ps_o{Q}", bufs=2, space="PSUM")
                    )
                    for h in range(HPQ):
                        g = Q * HPQ + h
                        gfi, gr = g // 2, (g % 2) * DH
                        r0 = (h % 2) * DH
                        ps_o = ps_o_pool.tile([DH + 1, LQS], F32, tag="ps_o")
                        for t in range(NKT):
                            ps_s = s_psum.tile([128, LQS], F32, tag="mm_s")
                            for c in range(2):
                                nc.tensor.matmul(
                                    ps_s[:, c * 512:(c + 1) * 512],
                                    khatT[h // 2][
                                        r0:r0 + DH, t * 128:(t + 1) * 128
                                    ].bitcast(F32R),
                                    qhatT[gfi][
                                        gr:gr + DH, c * 512:(c + 1) * 512
                                    ].bitcast(F32R),
                                    start=True,
                                    stop=True,
                                )
                            es = exp_pool.tile([128, LQS], F32R, tag="es")
                            nc.scalar.activation(
                                out=es[:], in_=ps_s[:], func=AF.Exp,
                                scale=inv_tau,
                            )
                            for c in range(2):
                                nc.tensor.matmul(
                                    ps_o[:, c * 512:(c + 1) * 512],
                                    vpp[:, t, h, :].bitcast(F32R),
                                    es[:, c * 512:(c + 1) * 512].bitcast(F32R),
                                    start=(t == 0),
                                    stop=(t == NKT - 1),
                                )
                        # normalize by the denominator row; broadcast the
                        # reciprocal across partitions with a K=1 PE matmul
                        # (ones[1,DH].T @ recd[1,LQS])
                        lnt = rec_pool.tile([1, LQS], F32, tag="lnt")
                        nc.scalar.activation(
                            out=lnt[:], in_=ps_o[DH:DH + 1, :], func=AF.Ln
                        )
                        recr = rec_pool.tile([1, LQS], F32R, tag="recr")
                        nc.scalar.activation(
                            out=recr[:], in_=lnt[:], func=AF.Exp, scale=-1.0
                        )
                        rb_ps = s_psum.tile([DH, LQS], F32, tag="mm_s")
                        for c in range(2):
                            nc.tensor.matmul(
                                rb_ps[:, c * 512:(c + 1) * 512],
                                ones_dh[:],
                                recr[0:1, c * 512:(c + 1) * 512],
                                start=True,
                                stop=True,
                            )
                        recb = rec_pool.tile([DH, LQS], F32, tag="recb")
                        nc.vector.tensor_copy(out=recb[:], in_=rb_ps[:])
                        nc.vector.tensor_tensor(
                            out=oaT[gfi][gr:gr + DH, :],
                            in0=ps_o[0:DH, :],
                            in1=recb[:],
                            op=ALU.mult,
                        )

            # ---- P3: output projection ----
            with contextlib.ExitStack() as p3:
                mm_psum = p3.enter_context(
                    tc.tile_pool(name="mm_psum3", bufs=3, space="PSUM")
                )
                wo_pool = p3.enter_context(tc.tile_pool(name="wo", bufs=1))
                wo_sb = wo_pool.tile([128, 2, NF, 512], F32R)
                for fi in range(NF):
                    for n in range(2):
                        nc.sync.dma_start(
                            out=wo_sb[:, n, fi, :],
                            in_=wo[fi * 128:(fi + 1) * 128,
                                   n * 512:(n + 1) * 512],
                        )
                fin_pool = p3.enter_context(tc.tile_pool(name="fin", bufs=3))
                for t in range(NQT):
                    ft = fin_pool.tile([128, DQ], F32, tag="fin")
                    for n in range(2):
                        ps = mm_psum.tile([128, 512], F32, tag="mm")
                        for fi in range(NF):
                            nc.tensor.matmul(
                                ps[:],
                                oaT[fi][:, t * 128:(t + 1) * 128
                                        ].bitcast(F32R),
                                wo_sb[:, n, fi, :].bitcast(F32R),
                                start=(fi == 0),
                                stop=(fi == NF - 1),
                            )
                        nc.vector.tensor_copy(
                            out=ft[:, n * 512:(n + 1) * 512], in_=ps[:]
                        )
                    nc.sync.dma_start(
                        out=out[t * 128:(t + 1) * 128, :], in_=ft[:]
                    )

    return nc


def split_multi_waits(nc):
    """walrus in this environment rejects >1 sync wait per instruction; move
    extras onto same-engine NOPs immediately preceding the instruction."""
    ctr = 0
    for f in nc.m.functions:
        for bb in f.blocks:
            new = []
            for inst in bb.instructions:
                si = inst.sync_info
                if si is not None and len(si.on_wait) > 1:
                    waits = list(si.on_wait)
                    for w in waits[:-1]:
                        nop = bass_rust.InstNoOp(name=f"I-wsplit-{ctr}")
                        ctr += 1
                        nop.engine = inst.engine
                        nop.sync_info = bass_rust.SyncInfo(
                            on_wait=[w], on_update=[]
                        )
                        new.append(nop)
                    inst.sync_info = bass_rust.SyncInfo(
                        on_wait=[waits[-1]], on_update=list(si.on_update)
                    )
                new.append(inst)
            bb.instructions[:] = new
    return ctr


_PROGRAM_CACHE = {}


def _get_program(inv_tau: float, has_bias: bool):
    key = (round(float(inv_tau), 12), has_bias)
    if key not in _PROGRAM_CACHE:
        nc = build_program(float(inv_tau), has_bias)
        split_multi_waits(nc)
        _PROGRAM_CACHE[key] = nc
    return _PROGRAM_CACHE[key]


def make_core_inputs(x, context, key_padding_mask, ln_q_w, ln_q_b, ln_ctx_w,
                     ln_ctx_b, Wq, Wk, Wv, Wo, tau):
    """Shard + host-side weight folding. Returns (in_maps, has_bias)."""
    f32 = np.float32
    x = np.asarray(x, f32)
    context = np.asarray(context, f32)
    mask01 = 1.0 - np.asarray(key_padding_mask).astype(f32)  # 1 = keep
    ln_q_w = np.asarray(ln_q_w, f32)
    ln_q_b = np.asarray(ln_q_b, f32)
    ln_ctx_w = np.asarray(ln_ctx_w, f32)
    ln_ctx_b = np.asarray(ln_ctx_b, f32)
    Wq = np.asarray(Wq, f32)
    Wk = np.asarray(Wk, f32)
    Wv = np.asarray(Wv, f32)
    Wo = np.asarray(Wo, f32)

    # fold LN affine into projections: (z*w + b) @ W = z @ (w*W) + b@W
    wq_f = np.ascontiguousarray(Wq * ln_q_w[:, None])
    wk_f = np.ascontiguousarray(Wk * ln_ctx_w[:, None])
    wv_f = np.ascontiguousarray(Wv * ln_ctx_w[:, None])
    has_bias = bool(np.any(ln_q_b != 0.0) or np.any(ln_ctx_b != 0.0))
    cq = (ln_q_b @ Wq).astype(f32)
    ck = (ln_ctx_b @ Wk).astype(f32)
    cv = (ln_ctx_b @ Wv).astype(f32)

    in_maps = []
    for core in range(8):
        b, hq = core // 2, core % 2
        m = {
            "xs": np.ascontiguousarray(x[b, hq * LQS:(hq + 1) * LQS, :]),
            "ctx": np.ascontiguousarray(context[b]),
            "mask01": np.ascontiguousarray(mask01[b]),
            "wq": wq_f,
            "wk": wk_f,
            "wv": wv_f,
            "wo": Wo,
        }
        if has_bias:
            m["cq"], m["ck"], m["cv"] = cq, ck, cv
        in_maps.append(m)
    return in_maps, has_bias


def kernel(x, context, key_padding_mask, ln_q_w, ln_q_b, ln_ctx_w, ln_ctx_b,
           Wq, Wk, Wv, Wo, tau, _trace=False):
    in_maps, has_bias = make_core_inputs(
        x, context, key_padding_mask, ln_q_w, ln_q_b, ln_ctx_w, ln_ctx_b,
        Wq, Wk, Wv, Wo, tau,
    )
    inv_tau = 1.0 / (float(np.asarray(tau)) + 1e-6)
    nc = _get_program(inv_tau, has_bias)
    res = run_bass_kernel_spmd(nc, in_maps, list(range(8)), trace=_trace)
    out = np.empty((B, LQ, DQ), np.float32)
    for core in range(8):
        b, hq = core // 2, core % 2
        out[b, hq * LQS:(hq + 1) * LQS, :] = res.results[core]["out"]
    if _trace:
        return out, res
    return out



# revision 6
# speedup vs baseline: 1.1049x; 1.1049x over previous
"""Trainium2 Bass kernel for nn_CrossAttnBlock (B=4, Lq=Lk=2048, D=1024, H=16, Dh=64).

Sharding: 8 cores = (batch b in 0..3) x (query half in 0..1). Each core computes
cross-attention for 1024 query tokens of one batch against that batch's full
2048-token context. K/V work is duplicated across the two cores of a batch
(zero collectives needed).

v2 (bf16): all matmul operands are bf16 (PSUM accumulation stays fp32), which
halves DMA traffic and SBUF so the LayerNormed context and all per-head K/V
tensors stay SBUF-resident (no DRAM staging). All token-major -> feature-major
transposes use the DMA crossbar (dma_start_transpose, 16-bit only), freeing the
PE from transpose matmuls and the DVE from PSUM eviction copies.

Per-core dataflow:
  P0: LayerNorm x, ctx token-major in bf16; DMA-xbar transpose into
      feature-major qinT / kvT ([128, 8, 2048] SBUF tiles).
  P1: q = qinT.T @ Wq (token-major), l2-normalize per head, xbar-transpose to
      feature-major qhatT.
  P2a: k/v projections for all 16 heads (kvT stationary, Wk/Wv stream);
      k l2-normalized token-major then xbar-transposed into kT (the natural
      [128, fi, ktok] layout packs head pairs exactly as attention needs);
      V'' = [v*mask | mask] built at evict (mask column preset once).
  P2b: per head: S^T[k,q] = khat.T @ qhat; exp(S^T/tau') on ACT straight to
      bf16; O' = V''.T @ expS^T accumulated over k tiles -> [65, q] with row
      64 = softmax denominator; normalize columns by 1/denom (DVE reciprocal +
      ones-matmul partition broadcast) into feature-major oaT.
  P3: out = oaT.T @ Wo (token-major, fp32), DMA to DRAM.

rsqrt is computed as exp(-0.5*ln(s)) + one Newton step so the whole kernel
needs a single ACT table set (natural_log_exp); ACT Rsqrt/Reciprocal are
banned for accuracy in this bass version. Softmax needs no max-subtraction:
q,k are l2-normalized so |scores/tau'| <= 8.01 even with bf16 rounding.
"""

import contextlib

import numpy as np

import bass_rust
import concourse.bass as bass
import concourse.tile as tile
from concourse import mybir
from concourse.bass_utils import run_bass_kernel_spmd

F32 = mybir.dt.float32
F32R = mybir.dt.float32r
BF16 = mybir.dt.bfloat16
AF = mybir.ActivationFunctionType
ALU = mybir.AluOpType

B, LQ, LK = 4, 2048, 2048
DQ, DC = 1024, 1024
H, DH = 16, 64
INNER = H * DH
LN_EPS = 1e-5

LQS = LQ // 2          # query tokens per core (1024)
NQT = LQS // 128       # 8 query token tiles
NKT = LK // 128        # 16 k token tiles
NF = DQ // 128         # 8 feature tiles


def _rsqrt(nc, pool, s_ap, out_ap, newton=True):
    """out = 1/sqrt(s) via exp(-0.5*ln(s)), optionally one Newton step.

    s_ap must be > 0. Shapes of s_ap/out_ap must match ([P, n])."""
    p, n = s_ap.shape[0], s_ap.free_size()
    t = pool.tile([p, n], F32, tag="rsq_t")
    nc.scalar.activation(out=t[:, :n], in_=s_ap, func=AF.Ln)
    nc.scalar.activation(out=out_ap, in_=t[:, :n], func=AF.Exp, scale=-0.5)
    if newton:
        # r = r * (1.5 - 0.5 * s * r^2)
        a = pool.tile([p, n], F32, tag="rsq_a")
        nc.vector.tensor_mul(out=a[:, :n], in0=out_ap, in1=out_ap)
        nc.vector.tensor_mul(out=a[:, :n], in0=a[:, :n], in1=s_ap)
        nc.vector.tensor_scalar(
            out=a[:, :n], in0=a[:, :n], scalar1=-0.5, scalar2=1.5,
            op0=ALU.mult, op1=ALU.add,
        )
        nc.vector.tensor_mul(out=out_ap, in0=out_ap, in1=a[:, :n])


def _layernorm_tile(nc, pool, x_ap, z_ap):
    """z = (x - mean) * rsqrt(var + eps) for a [128, 1024] token-major tile."""
    p = x_ap.shape[0]
    stats = pool.tile([p, 2, 6], F32, tag="ln_stats")
    x3 = x_ap.rearrange("p (c f) -> p c f", c=2)
    for c in range(2):
        nc.vector.bn_stats(out=stats[:, c, :], in_=x3[:, c, :])
    mv = pool.tile([p, 2], F32, tag="ln_mv")
    nc.vector.bn_aggr(out=mv[:], in_=stats[:])
    s = pool.tile([p, 1], F32, tag="ln_s")
    nc.vector.tensor_scalar(
        out=s[:], in0=mv[:, 1:2], scalar1=LN_EPS, scalar2=None, op0=ALU.add,
    )
    inv = pool.tile([p, 1], F32, tag="ln_inv")
    _rsqrt(nc, pool, s[:], inv[:])
    nc.vector.tensor_scalar(
        out=z_ap, in0=x_ap, scalar1=mv[:, 0:1], scalar2=inv[:],
        op0=ALU.subtract, op1=ALU.mult,
    )


def build_program(inv_tau: float, has_bias: bool):
    """Build the single-core SPMD bass program."""
    nc = bass.Bass()

    xs = nc.declare_dram_parameter("xs", [LQS, DQ], BF16, isOutput=False)
    ctx = nc.declare_dram_parameter("ctx", [LK, DC], BF16, isOutput=False)
    mask01 = nc.declare_dram_parameter("mask01", [LK], F32, isOutput=False)
    wq = nc.declare_dram_parameter("wq", [DQ, INNER], BF16, isOutput=False)
    wk = nc.declare_dram_parameter("wk", [DC, INNER], BF16, isOutput=False)
    wv = nc.declare_dram_parameter("wv", [DC, INNER], BF16, isOutput=False)
    wo = nc.declare_dram_parameter("wo", [INNER, DQ], BF16, isOutput=False)
    if has_bias:
        cq = nc.declare_dram_parameter("cq", [INNER], F32, isOutput=False)
        ck = nc.declare_dram_parameter("ck", [INNER], F32, isOutput=False)
        cv = nc.declare_dram_parameter("cv", [INNER], F32, isOutput=False)
    out = nc.declare_dram_parameter("out", [LQS, DQ], F32, isOutput=True)

    with tile.TileContext(nc) as tc:
        with contextlib.ExitStack() as stack:
            consts = stack.enter_context(tc.tile_pool(name="consts", bufs=1))
            ones_f = consts.tile([1, DH], F32)
            nc.vector.memset(ones_f[:], 1.0)
            ones_r = consts.tile([1, DH], F32R)
            nc.vector.tensor_copy(out=ones_r[:], in_=ones_f[:])
            ones_dh = ones_r[:]

            # mask as [128, NKT] float (token t*128+p at [p, t])
            mask_sb = consts.tile([128, NKT], F32)
            nc.sync.dma_start(
                out=mask_sb[:], in_=mask01.rearrange("(t p) -> p t", p=128)
            )
            if has_bias:
                # per-out-feature projection bias rows broadcast to 128 parts
                cq_b = consts.tile([128, INNER], F32)
                ck_b = consts.tile([128, INNER], F32)
                cv_b = consts.tile([128, INNER], F32)
                for dst, src in ((cq_b, cq), (ck_b, ck), (cv_b, cv)):
                    bc = bass.AP(
                        tensor=src.tensor, offset=src.offset,
                        ap=[[0, 128]] + list(src.ap),
                    )
                    nc.gpsimd.dma_start(out=dst[:], in_=bc)

            small = stack.enter_context(tc.tile_pool(name="small", bufs=2))

            # persistent feature-major activations (bf16)
            persist = stack.enter_context(tc.tile_pool(name="persist", bufs=1))
            qhatT = persist.tile([128, NF, LQS], BF16, name="qhatT")
            oaT = persist.tile([128, NF, LQS], BF16, name="oaT")

            # kT/vpp persist into P2b; entered before the kvin era so pools
            # close in stack (LIFO) order.
            kvw_pool = stack.enter_context(tc.tile_pool(name="kvw", bufs=1))
            kT = kvw_pool.tile([128, NF, LK], BF16, name="kT")
            vpp = kvw_pool.tile([128, NKT, H, DH + 1], BF16, name="vpp")
            # preset all mask columns once: vpp[:, t, h, 64] = mask[:, t]
            nc.gpsimd.tensor_copy(
                out=vpp[:, :, :, DH:DH + 1],
                in_=mask_sb[:].unsqueeze(2).broadcast_to([128, NKT, H]
                                                        ).unsqueeze(3),
            )

            # qinT/kvT live P0 -> P2a (own pool so they free before P2b)
            with contextlib.ExitStack() as kv_era:
                kvin_pool = kv_era.enter_context(
                    tc.tile_pool(name="kvin", bufs=1)
                )
                qinT = kvin_pool.tile([128, NF, LQS], BF16, name="qinT")
                kvT = kvin_pool.tile([128, NF, LK], BF16, name="kvT")

                # ---- P0: LayerNorm + xbar transpose ----
                with contextlib.ExitStack() as p0:
                    ln_pool = p0.enter_context(tc.tile_pool(name="ln", bufs=3))
                    for t in range(NQT):
                        xt = ln_pool.tile([128, DQ], BF16, tag="ln_x")
                        nc.sync.dma_start(
                            out=xt[:], in_=xs[t * 128:(t + 1) * 128, :]
                        )
                        zt = ln_pool.tile([128, DQ], BF16, tag="ln_z")
                        _layernorm_tile(nc, small, xt[:], zt[:])
                        nc.sync.dma_start_transpose(
                            out=qinT[:, :, t * 128:(t + 1) * 128], in_=zt[:]
                        )
                    for t in range(NKT):
                        xt = ln_pool.tile([128, DQ], BF16, tag="ln_x")
                        nc.sync.dma_start(
                            out=xt[:], in_=ctx[t * 128:(t + 1) * 128, :]
                        )
                        zt = ln_pool.tile([128, DQ], BF16, tag="ln_z")
                        _layernorm_tile(nc, small, xt[:], zt[:])
                        nc.sync.dma_start_transpose(
                            out=kvT[:, :, t * 128:(t + 1) * 128], in_=zt[:]
                        )

                # ---- P1: q projection, l2 norm, xbar transpose ----
                with contextlib.ExitStack() as p1:
                    mm_psum = p1.enter_context(
                        tc.tile_pool(name="mm_psum1", bufs=3, space="PSUM")
                    )
                    wq_pool = p1.enter_context(tc.tile_pool(name="wq", bufs=1))
                    wq_sb = wq_pool.tile([128, NF, INNER], BF16)
                    for fi in range(NF):
                        nc.sync.dma_start(
                            out=wq_sb[:, fi, :],
                            in_=wq[fi * 128:(fi + 1) * 128, :],
                        )
                    qt_pool = p1.enter_context(tc.tile_pool(name="qtok", bufs=2))
                    for t in range(NQT):
                        qtok = qt_pool.tile([128, INNER], BF16, tag="qtok")
                        for n in range(2):
                            ps = mm_psum.tile([128, 512], F32, tag="mm")
                            for fi in range(NF):
                                nc.tensor.matmul(
                                    ps[:],
                                    qinT[:, fi, t * 128:(t + 1) * 128],
                                    wq_sb[:, fi, n * 512:(n + 1) * 512],
                                    start=(fi == 0),
                                    stop=(fi == NF - 1),
                                )
                            if has_bias:
                                nc.vector.tensor_add(
                                    out=qtok[:, n * 512:(n + 1) * 512],
                                    in0=ps[:],
                                    in1=cq_b[:, n * 512:(n + 1) * 512],
                                )
                            else:
                                nc.scalar.copy(
                                    out=qtok[:, n * 512:(n + 1) * 512],
                                    in_=ps[:],
                                )
                        # ssq per head -> rsqrt -> normalize in place
                        sq = qt_pool.tile([128, INNER], BF16, tag="qsq")
                        nc.vector.tensor_mul(out=sq[:], in0=qtok[:], in1=qtok[:])
                        ssq = small.tile([128, H], F32, tag="qssq")
                        nc.vector.tensor_reduce(
                            out=ssq[:],
                            in_=sq[:].rearrange("p (h d) -> p h d", h=H),
                            axis=mybir.AxisListType.X,
                            op=ALU.add,
                        )
                        rq = small.tile([128, H], F32, tag="qrq")
                        _rsqrt(nc, small, ssq[:], rq[:])
                        q3 = qtok[:].rearrange("p (h d) -> p h d", h=H)
                        nc.vector.tensor_tensor(
                            out=q3,
                            in0=q3,
                            in1=rq[:].unsqueeze(2).broadcast_to([128, H, DH]),
                            op=ALU.mult,
                        )
                        nc.sync.dma_start_transpose(
                            out=qhatT[:, :, t * 128:(t + 1) * 128], in_=qtok[:]
                        )

                # ---- P2a: k/v projection, two half-passes of 8 heads ----
                # kT[:, fi, :] holds features [fi*128, (fi+1)*128) = heads
                # (2*fi, 2*fi+1) stacked 64+64 on partitions: exactly the
                # head-pair packing attention wants.
                with contextlib.ExitStack() as p2a:
                    wkv_pool = p2a.enter_context(
                        tc.tile_pool(name="wkv", bufs=2)
                    )
                    mm_psum = p2a.enter_context(
                        tc.tile_pool(name="mm_psum2", bufs=2, space="PSUM")
                    )
                    kt_pool = p2a.enter_context(
                        tc.tile_pool(name="ktok", bufs=3)
                    )
                    HH = H // 2  # heads per half-pass (8)
                    for n in range(2):
                        c0 = n * 512
                        wk_sb = wkv_pool.tile([128, NF, 512], BF16, tag="wk_h")
                        wv_sb = wkv_pool.tile([128, NF, 512], BF16, tag="wv_h")
                        for fi in range(NF):
                            nc.sync.dma_start(
                                out=wk_sb[:, fi, :],
                                in_=wk[fi * 128:(fi + 1) * 128, c0:c0 + 512],
                            )
                            nc.sync.dma_start(
                                out=wv_sb[:, fi, :],
                                in_=wv[fi * 128:(fi + 1) * 128, c0:c0 + 512],
                            )
                        for t in range(NKT):
                            ps_k = mm_psum.tile([128, 512], F32, tag="mm_k")
                            ps_v = mm_psum.tile([128, 512], F32, tag="mm_v")
                            for fi in range(NF):
                                lhs = kvT[:, fi, t * 128:(t + 1) * 128]
                                nc.tensor.matmul(
                                    ps_k[:], lhs, wk_sb[:, fi, :],
                                    start=(fi == 0), stop=(fi == NF - 1),
                                )
                                nc.tensor.matmul(
                                    ps_v[:], lhs, wv_sb[:, fi, :],
                                    start=(fi == 0), stop=(fi == NF - 1),
                                )
                            ktok = kt_pool.tile([128, 512], BF16, tag="ktok")
                            if has_bias:
                                nc.vector.tensor_add(
                                    out=ktok[:], in0=ps_k[:],
                                    in1=ck_b[:, c0:c0 + 512],
                                )
                            else:
                                nc.scalar.copy(out=ktok[:], in_=ps_k[:])
                            # V'' = [v * mask | mask] (mask col preset above)
                            mt = mask_sb[:, t:t + 1]
                            if has_bias:
                                vtmp = kt_pool.tile([128, 512], F32,
                                                    tag="vtmp")
                                nc.vector.tensor_add(
                                    out=vtmp[:], in0=ps_v[:],
                                    in1=cv_b[:, c0:c0 + 512],
                                )
                                vsrc = vtmp[:]
                            else:
                                vsrc = ps_v[:]
                            nc.vector.tensor_scalar_mul(
                                out=vpp[:, t, n * HH:(n + 1) * HH, 0:DH],
                                in0=vsrc.rearrange("p (h d) -> p h d", h=HH),
                                scalar1=mt,
                            )
                            # k l2 norm (in place) then xbar transpose
                            sqk = kt_pool.tile([128, 512], BF16, tag="ksq")
                            nc.vector.tensor_mul(
                                out=sqk[:], in0=ktok[:], in1=ktok[:]
                            )
                            ssqk = small.tile([128, HH], F32, tag="kssq")
                            nc.vector.tensor_reduce(
                                out=ssqk[:],
                                in_=sqk[:].rearrange("p (h d) -> p h d", h=HH),
                                axis=mybir.AxisListType.X,
                                op=ALU.add,
                            )
                            rk = small.tile([128, HH], F32, tag="krk")
                            _rsqrt(nc, small, ssqk[:], rk[:])
                            k3 = ktok[:].rearrange("p (h d) -> p h d", h=HH)
                            nc.vector.tensor_tensor(
                                out=k3,
                                in0=k3,
                                in1=rk[:].unsqueeze(2).broadcast_to(
                                    [128, HH, DH]
                                ),
                                op=ALU.mult,
                            )
                            nc.sync.dma_start_transpose(
                                out=kT[:, 4 * n:4 * (n + 1),
                                       t * 128:(t + 1) * 128],
                                in_=ktok[:],
                            )

            # ---- P2b: attention per head ----
            with contextlib.ExitStack() as p2b:
                exp_pool = p2b.enter_context(
                    tc.tile_pool(name="exp", bufs=4)
                )
                rec_pool = p2b.enter_context(
                    tc.tile_pool(name="rec", bufs=2)
                )
                s_psum = p2b.enter_context(
                    tc.tile_pool(name="s_psum", bufs=2, space="PSUM")
                )
                ps_o_pool = p2b.enter_context(
                    tc.tile_pool(name="ps_o", bufs=2, space="PSUM")
                )
                for g in range(H):
                    gfi, gr = g // 2, (g % 2) * DH
                    ps_o = ps_o_pool.tile([DH + 1, LQS], F32, tag="ps_o")
                    for t in range(NKT):
                        ps_s = s_psum.tile([128, LQS], F32, tag="mm_s")
                        for c in range(2):
                            nc.tensor.matmul(
                                ps_s[:, c * 512:(c + 1) * 512],
                                kT[gr:gr + DH, gfi, t * 128:(t + 1) * 128],
                                qhatT[gr:gr + DH, gfi,
                                      c * 512:(c + 1) * 512],
                                start=True,
                                stop=True,
                            )
                        es = exp_pool.tile([128, LQS], BF16, tag="es")
                        nc.scalar.activation(
                            out=es[:], in_=ps_s[:], func=AF.Exp,
                            scale=inv_tau,
                        )
                        for c in range(2):
                            nc.tensor.matmul(
                                ps_o[:, c * 512:(c + 1) * 512],
                                vpp[:, t, g, :],
                                es[:, c * 512:(c + 1) * 512],
                                start=(t == 0),
                                stop=(t == NKT - 1),
                            )
                    # normalize by the denominator row; broadcast the
                    # reciprocal across partitions with a K=1 PE matmul
                    # (ones[1,DH].T @ recd[1,LQS])
                    rec_f = rec_pool.tile([1, LQS], F32, tag="rec_f")
                    nc.vector.reciprocal(
                        out=rec_f[:], in_=ps_o[DH:DH + 1, :]
                    )
                    recr = rec_pool.tile([1, LQS], F32R, tag="recr")
                    nc.vector.tensor_copy(out=recr[:], in_=rec_f[:])
                    rb_ps = s_psum.tile([DH, LQS], F32, tag="mm_s")
                    for c in range(2):
                        nc.tensor.matmul(
                            rb_ps[:, c * 512:(c + 1) * 512],
                            ones_dh,
                            recr[0:1, c * 512:(c + 1) * 512],
                            start=True,
                            stop=True,
                        )
                    recb = rec_pool.tile([DH, LQS], F32, tag="recb")
                    nc.vector.tensor_copy(out=recb[:], in_=rb_ps[:])
                    nc.vector.tensor_tensor(
                        out=oaT[gr:gr + DH, gfi, :],
                        in0=ps_o[0:DH, :],
                        in1=recb[:],
                        op=ALU.mult,
                    )

            # ---- P3: output projection ----
            with contextlib.ExitStack() as p3:
                mm_psum = p3.enter_context(
                    tc.tile_pool(name="mm_psum3", bufs=3, space="PSUM")
                )
                wo_pool = p3.enter_context(tc.tile_pool(name="wo", bufs=1))
                wo_sb = wo_pool.tile([128, NF, DQ], BF16)
                for fi in range(NF):
                    nc.sync.dma_start(
                        out=wo_sb[:, fi, :],
                        in_=wo[fi * 128:(fi + 1) * 128, :],
                    )
                fin_pool = p3.enter_context(tc.tile_pool(name="fin", bufs=3))
                for t in range(NQT):
                    ft = fin_pool.tile([128, DQ], F32, tag="fin")
                    for n in range(2):
                        ps = mm_psum.tile([128, 512], F32, tag="mm")
                        for fi in range(NF):
                            nc.tensor.matmul(
                                ps[:],
                                oaT[:, fi, t * 128:(t + 1) * 128],
                                wo_sb[:, fi, n * 512:(n + 1) * 512],
                                start=(fi == 0),
                                stop=(fi == NF - 1),
                            )
                        nc.scalar.copy(
                            out=ft[:, n * 512:(n + 1) * 512], in_=ps[:]
                        )
                    nc.sync.dma_start(
                        out=out[t * 128:(t + 1) * 128, :], in_=ft[:]
                    )

    return nc


def split_multi_waits(nc):
    """walrus in this environment rejects >1 sync wait per instruction; move
    extras onto same-engine NOPs immediately preceding the instruction."""
    ctr = 0
    for f in nc.m.functions:
        for bb in f.blocks:
            new = []
            for inst in bb.instructions:
                si = inst.sync_info
                if si is not None and len(si.on_wait) > 1:
                    waits = list(si.on_wait)
                    for w in waits[:-1]:
                        nop = bass_rust.InstNoOp(name=f"I-wsplit-{ctr}")
                        ctr += 1
                        nop.engine = inst.engine
                        nop.sync_info = bass_rust.SyncInfo(
                            on_wait=[w], on_update=[]
                        )
                        new.append(nop)
                    inst.sync_info = bass_rust.SyncInfo(
                        on_wait=[waits[-1]], on_update=list(si.on_update)
                    )
                new.append(inst)
            bb.instructions[:] = new
    return ctr


_PROGRAM_CACHE = {}


def _get_program(inv_tau: float, has_bias: bool):
    key = (round(float(inv_tau), 12), has_bias)
    if key not in _PROGRAM_CACHE:
        nc = build_program(float(inv_tau), has_bias)
        split_multi_waits(nc)
        _PROGRAM_CACHE[key] = nc
    return _PROGRAM_CACHE[key]


def make_core_inputs(x, context, key_padding_mask, ln_q_w, ln_q_b, ln_ctx_w,
                     ln_ctx_b, Wq, Wk, Wv, Wo, tau):
    """Shard + host-side weight folding. Returns (in_maps, has_bias)."""
    import ml_dtypes

    f32 = np.float32
    bf16 = ml_dtypes.bfloat16
    x = np.asarray(x, f32)
    context = np.asarray(context, f32)
    mask01 = 1.0 - np.asarray(key_padding_mask).astype(f32)  # 1 = keep
    ln_q_w = np.asarray(ln_q_w, f32)
    ln_q_b = np.asarray(ln_q_b, f32)
    ln_ctx_w = np.asarray(ln_ctx_w, f32)
    ln_ctx_b = np.asarray(ln_ctx_b, f32)
    Wq = np.asarray(Wq, f32)
    Wk = np.asarray(Wk, f32)
    Wv = np.asarray(Wv, f32)
    Wo = np.asarray(Wo, f32)

    # fold LN affine into projections: (z*w + b) @ W = z @ (w*W) + b@W
    wq_f = np.ascontiguousarray(Wq * ln_q_w[:, None]).astype(bf16)
    wk_f = np.ascontiguousarray(Wk * ln_ctx_w[:, None]).astype(bf16)
    wv_f = np.ascontiguousarray(Wv * ln_ctx_w[:, None]).astype(bf16)
    wo_f = Wo.astype(bf16)
    has_bias = bool(np.any(ln_q_b != 0.0) or np.any(ln_ctx_b != 0.0))
    cq = (ln_q_b @ Wq).astype(f32)
    ck = (ln_ctx_b @ Wk).astype(f32)
    cv = (ln_ctx_b @ Wv).astype(f32)

    x_b = x.astype(bf16)
    ctx_b = context.astype(bf16)

    in_maps = []
    for core in range(8):
        b, hq = core // 2, core % 2
        m = {
            "xs": np.ascontiguousarray(x_b[b, hq * LQS:(hq + 1) * LQS, :]),
            "ctx": np.ascontiguousarray(ctx_b[b]),
            "mask01": np.ascontiguousarray(mask01[b]),
            "wq": wq_f,
            "wk": wk_f,
            "wv": wv_f,
            "wo": wo_f,
        }
        if has_bias:
            m["cq"], m["ck"], m["cv"] = cq, ck, cv
        in_maps.append(m)
    return in_maps, has_bias


def kernel(x, context, key_padding_mask, ln_q_w, ln_q_b, ln_ctx_w, ln_ctx_b,
           Wq, Wk, Wv, Wo, tau, _trace=False):
    in_maps, has_bias = make_core_inputs(
        x, context, key_padding_mask, ln_q_w, ln_q_b, ln_ctx_w, ln_ctx_b,
        Wq, Wk, Wv, Wo, tau,
    )
    inv_tau = 1.0 / (float(np.asarray(tau)) + 1e-6)
    nc = _get_program(inv_tau, has_bias)
    res = run_bass_kernel_spmd(nc, in_maps, list(range(8)), trace=_trace)
    out = np.empty((B, LQ, DQ), np.float32)
    for core in range(8):
        b, hq = core // 2, core % 2
        out[b, hq * LQS:(hq + 1) * LQS, :] = res.results[core]["out"]
    if _trace:
        return out, res
    return out
